# revision 17
# baseline (speedup 1.0000x reference)
"""AGCN (Chebyshev graph conv + per-node clustered GEMM + bias + cluster-max)
distributed over 8 trn2 NeuronCores.

Full inputs in, full output out. Internally:
  - node dim (420) sharded across 8 cores (52/53 nodes each, padded to 53)
  - x replicated to every core (host-side staging; aggregation over m needs full x)
  - per-core Bass kernel:
      S1 = L, S2 = 2 L @ L - I           (computed on device, transposed layout)
      xgT[k][c, b, n] = sum_m S_k[n, m] x[b, m, c]   (b-pair matmuls, x stationary)
      out[b, n, o]   = max_cl( sum_{k,i} xgT[k][i,b,n] W[n,k,i,o,cl] + bias[n,o,cl] )
    with W streamed from HBM (memory-bound term), bias fused via a ones-row,
    and the cluster max fused into the PSUM eviction.
"""

import sys

for _p in ("/opt/trn_rl_repo",):
    if _p not in sys.path:
        sys.path.insert(0, _p)

import numpy as np

# ---------------- problem constants (hardcoded) ----------------
B = 64         # batch
NN = 420       # nodes
C = 64         # dim_in
O = 64         # dim_out
CL = 10        # cluster dim
KCH = 3        # chebyshev order
NCORES = 8
PN = 53        # padded per-core node count
BOUNDS = [(NN * i) // NCORES for i in range(NCORES + 1)]
OCL = O * CL           # 640
KI = KCH * C           # 192
ROWS = KI + 1          # 193: 192 weight rows + 1 bias row
BC = B * C             # 4096
MCH = [128, 128, 128, 36]   # contraction (m / j) chunk sizes covering 420
NQ = (PN + 1) // 2     # 27 node-pair blocks in the output buffer
W2B = 32               # partition base for the 65-row chunk-2 tiles

USE_BF16 = True        # compute/storage dtype for matmul operands


def _np_dt():
    if USE_BF16:
        import ml_dtypes

        return np.dtype(ml_dtypes.bfloat16)
    return np.dtype(np.float32)


# ---------------- device graph ----------------
_NC_CACHE = None


def _build():
    global _NC_CACHE
    if _NC_CACHE is not None:
        return _NC_CACHE

    from contextlib import ExitStack

    import concourse.bacc as bacc
    import concourse.mybir as mybir
    import concourse.tile as tile

    DT = mybir.dt.bfloat16 if USE_BF16 else mybir.dt.float32
    F32 = mybir.dt.float32

    nc = bacc.Bacc(
        "TRN2",
        target_bir_lowering=False,
        debug=False,
        enable_asserts=False,
        num_devices=NCORES,
    )

    xt_d = nc.dram_tensor("xt", [128, 4 * BC], DT, kind="ExternalInput")
    xloc_d = nc.dram_tensor("xloc", [C, B * PN], DT, kind="ExternalInput")
    l2_d = nc.dram_tensor("l2", [128, 4 * NN], DT, kind="ExternalInput")
    lt_d = nc.dram_tensor("lt", [128, 4 * PN], DT, kind="ExternalInput")
    it_d = nc.dram_tensor("it", [128, 4 * PN], F32, kind="ExternalInput")
    w_d = nc.dram_tensor("w", [PN, ROWS, OCL], DT, kind="ExternalInput")
    # out[b, parity, q, o] -> node 2q+parity
    out_d = nc.dram_tensor("out", [B, 2, NQ, O], F32, kind="ExternalOutput")

    # contraction sub-chunks (chunk idx, base row within chunk, rows):
    # 64-row pieces so consecutive LDWEIGHTS target alternating row groups
    SUBS = [(mc, 0, MCH[mc]) for mc in range(4)]

    with tile.TileContext(nc) as tc:
        with ExitStack() as ctx:
            const = ctx.enter_context(tc.tile_pool(name="const", bufs=1))
            w1p = ctx.enter_context(tc.tile_pool(name="w1p", bufs=11))
            w2p = ctx.enter_context(tc.tile_pool(name="w2p", bufs=11))
            ps2p = ctx.enter_context(tc.tile_pool(name="ps2", bufs=2, space="PSUM"))
            ps3p = ctx.enter_context(tc.tile_pool(name="ps3", bufs=3, space="PSUM"))

            xt = const.tile([128, 4 * BC], DT)          # x as [m, (b c)], 4 m-chunks
            sT = const.tile([128, 4 * 106], DT)         # per chunk: [S1T | S2T] cols
            l2 = const.tile([128, 4 * NN], DT)          # 2L as [j, m], 4 j-chunks
            ilocT = const.tile([128, 4 * PN], F32)      # I_loc^T, 4 m-chunks
            xg01 = const.tile([128, B * PN], DT)        # rows: k=0 (c) | k=1 (c)
            xg2 = const.tile([65, B * PN], DT)          # rows 0..63: k=2, row 64: ones
            NQA = 14
            outbA = const.tile([128, NQA * O], F32)     # pairs q < 14
            outbB = const.tile([128, (NQ - NQA) * O], F32)  # pairs q >= 14

            # ---- phase 0: load x, L-derived tensors (packed, one DMA each) ----
            nc.sync.dma_start(
                out=sT[:, :].rearrange("p (c x) -> p c x", x=106)[:, :, 0:PN],
                in_=lt_d[:, :].rearrange("p (c x) -> p c x", x=PN),
            )
            nc.sync.dma_start(out=l2[:, :], in_=l2_d[:, :])
            nc.sync.dma_start(out=ilocT[:, :], in_=it_d[:, :])
            for c in range(4):
                nc.sync.dma_start(
                    out=xt[:, c * BC : (c + 1) * BC],
                    in_=xt_d[:, c * BC : (c + 1) * BC],
                )
            nc.scalar.dma_start(out=xg01[0:C, :], in_=xloc_d[:, :])
            nc.gpsimd.memset(xg2[64:65, :], 1.0)

            # ---- phase 1: S2T[m, n] = 2 (L @ L)[n_glob, m] - I ----
            for mc in range(4):
                m = MCH[mc]
                ps = ps2p.tile([128, 424], F32)
                for jc in range(4):
                    nc.tensor.matmul(
                        ps[0:m, 0:PN],
                        lhsT=l2[0:128, jc * NN + mc * 128 : jc * NN + mc * 128 + m],
                        rhs=sT[0:128, jc * 106 : jc * 106 + PN],
                        start=(jc == 0),
                        stop=(jc == 3),
                    )
                nc.vector.tensor_sub(
                    sT[0:m, mc * 106 + PN : mc * 106 + 106],
                    ps[0:m, 0:PN],
                    ilocT[0:m, mc * PN : (mc + 1) * PN],
                )

            # ---- phase 2: xgT for k=1,2 via b-pair matmuls (x stationary) ----
            # 4 b-pairs share one single-bank psum tile; evictions are 4 wide
            # strided CASTs per group instead of 16 narrow ones.
            xg01w = xg01[:, :].rearrange("p (pb two n) -> p pb two n", two=2, n=PN)
            xg2w = xg2[:, :].rearrange("p (pb two n) -> p pb two n", two=2, n=PN)
            for g in range(B // 8):          # 8 groups of 4 b-pairs
                p0 = g * 4
                ps = ps2p.tile([128, 424], F32)
                for j in range(4):
                    p = p0 + j
                    nsub = len(SUBS)
                    for si, (mc, b0, sz) in enumerate(SUBS):
                        nc.tensor.matmul(
                            ps[:, j * 106 : j * 106 + 106],
                            lhsT=xt[
                                b0 : b0 + sz, mc * BC + p * 128 : mc * BC + (p + 1) * 128
                            ],
                            rhs=sT[b0 : b0 + sz, mc * 106 : mc * 106 + 106],
                            start=(si == 0),
                            stop=(si == nsub - 1),
                        )
                psw = ps[:, :].rearrange("p (j x) -> p j x", x=106)
                for h in range(2):
                    nc.vector.tensor_copy(
                        xg01w[64:128, p0 : p0 + 4, h, :],
                        psw[h * 64 : (h + 1) * 64, :, 0:PN],
                    )
                    nc.vector.tensor_copy(
                        xg2w[0:64, p0 : p0 + 4, h, :],
                        psw[h * 64 : (h + 1) * 64, :, PN:106],
                    )

            # ---- phase 3: per-node GEMM + bias + cluster max ----
            xg01v = xg01[:, :].rearrange("p (b n) -> p b n", n=PN)
            xg2v = xg2[:, :].rearrange("p (b n) -> p b n", n=PN)

            GN = 4
            groups = [(g * GN, min(GN, PN - g * GN)) for g in range((PN + GN - 1) // GN)]
            for n0, gs in groups:
                w1 = w1p.tile([128, 4 * OCL], DT)
                w2 = w2p.tile([65, 4 * OCL], DT)
                nc.sync.dma_start(
                    out=w1[0:128, 0 : gs * OCL].rearrange("r (n f) -> r n f", f=OCL),
                    in_=w_d[n0 : n0 + gs, 0:128, :].rearrange("n r f -> r n f"),
                )
                nc.scalar.dma_start(
                    out=w2[0:65, 0 : gs * OCL].rearrange("r (n f) -> r n f", f=OCL),
                    in_=w_d[n0 : n0 + gs, 128:ROWS, :].rearrange("n r f -> r n f"),
                )
                local = 0
                while local < gs:
                    npair = 2 if local + 1 < gs else 1
                    ps = ps3p.tile([128, OCL], F32)
                    for t in range(npair):
                        node = n0 + local + t
                        gi = local + t
                        tp = None if t == 0 else (0, 64)
                        pr = slice(64 * t, 64 * (t + 1))
                        l1 = xg01v[0:128, :, node : node + 1]
                        l2h = xg2v[0:65, :, node : node + 1]
                        r1 = w1[0:128, gi * OCL : (gi + 1) * OCL]
                        r2 = w2[0:65, gi * OCL : (gi + 1) * OCL]
                        nc.tensor.matmul(
                            ps[pr, 0:512], lhsT=l1, rhs=r1[:, 0:512],
                            start=True, stop=False, tile_position=tp,
                        )
                        nc.tensor.matmul(
                            ps[pr, 512:OCL], lhsT=l1, rhs=r1[:, 512:OCL],
                            start=True, stop=False, tile_position=tp,
                        )
                        nc.tensor.matmul(
                            ps[pr, 0:512], lhsT=l2h, rhs=r2[:, 0:512],
                            start=False, stop=True, tile_position=tp,
                        )
                        nc.tensor.matmul(
                            ps[pr, 512:OCL], lhsT=l2h, rhs=r2[:, 512:OCL],
                            start=False, stop=True, tile_position=tp,
                        )
                    q = (n0 + local) // 2
                    pp = 64 * npair
                    ob, qq = (outbA, q) if q < NQA else (outbB, q - NQA)
                    nc.vector.reduce_max(
                        ob[0:pp, qq * O : (qq + 1) * O],
                        ps[0:pp, :].rearrange("p (o c) -> p o c", c=CL),
                        axis=mybir.AxisListType.X,
                    )
                    local += npair

            # ---- phase 4: write out (A-half can start while B still computes) ----
            nc.scalar.dma_start(
                out=out_d[:, 0, 0:NQA, :],
                in_=outbA[0:64, :].rearrange("p (q o) -> p q o", o=O),
            )
            nc.scalar.dma_start(
                out=out_d[:, 1, 0:NQA, :],
                in_=outbA[64:128, :].rearrange("p (q o) -> p q o", o=O),
            )
            nc.scalar.dma_start(
                out=out_d[:, 0, NQA:NQ, :],
                in_=outbB[0:64, :].rearrange("p (q o) -> p q o", o=O),
            )
            nc.scalar.dma_start(
                out=out_d[:, 1, NQA : NQ - 1, :],
                in_=outbB[64:128, 0 : (NQ - 1 - NQA) * O].rearrange(
                    "p (q o) -> p q o", o=O
                ),
            )

    nc.compile()
    _NC_CACHE = nc
    return nc


# ---------------- host-side sharding / staging ----------------
def prepare_in_maps(x, node_embeddings, laplacian_mx, cluster_weights_pool, bias_pool):
    x = np.ascontiguousarray(np.asarray(x, dtype=np.float32))
    L = np.ascontiguousarray(np.asarray(laplacian_mx, dtype=np.float32))
    W = np.asarray(cluster_weights_pool, dtype=np.float32)
    bias = np.asarray(bias_pool, dtype=np.float32)
    dt = _np_dt()

    def _pack(a):
        # [420, F] -> [128, 4*F] chunk-major with zero padding to 512 rows
        f = a.shape[1]
        p = np.zeros((512, f), dtype=a.dtype)
        p[:NN] = a
        return np.ascontiguousarray(
            p.reshape(4, 128, f).transpose(1, 0, 2).reshape(128, 4 * f)
        )

    xt = _pack(x.transpose(1, 0, 2).reshape(NN, BC)).astype(dt)
    l2 = _pack(2.0 * L).astype(dt)
    Wr = W.reshape(NN, KI, OCL)
    br = bias.reshape(NN, OCL)

    in_maps = []
    for i in range(NCORES):
        o0, o1 = BOUNDS[i], BOUNDS[i + 1]
        ni = o1 - o0
        xloc = np.zeros((C, B, PN), dtype=np.float32)
        xloc[:, :, :ni] = x[:, o0:o1, :].transpose(2, 0, 1)
        lt = np.zeros((NN, PN), dtype=np.float32)
        lt[:, :ni] = L[o0:o1, :].T
        lt = _pack(lt)
        it = np.zeros((NN, PN), dtype=np.float32)
        it[np.arange(o0, o1), np.arange(ni)] = 1.0
        it = _pack(it)
        w = np.zeros((PN, ROWS, OCL), dtype=np.float32)
        w[:ni, :KI] = Wr[o0:o1]
        w[:ni, KI] = br[o0:o1]
        in_maps.append(
            {
                "xt": xt,
                "xloc": np.ascontiguousarray(xloc.reshape(C, B * PN)).astype(dt),
                "l2": l2,
                "lt": lt.astype(dt),
                "it": it,
                "w": w.astype(dt),
            }
        )
    return in_maps


def run_device(in_maps, trace=False, **kwargs):
    from concourse.bass_utils import run_bass_kernel_spmd

    nc = _build()
    return run_bass_kernel_spmd(
        nc, in_maps, core_ids=list(range(NCORES)), trace=trace, **kwargs
    )


def assemble(results):
    out = np.zeros((B, NN, O), dtype=np.float32)
    for i in range(NCORES):
        o0, o1 = BOUNDS[i], BOUNDS[i + 1]
        ni = o1 - o0
        arr = np.asarray(results[i]["out"], dtype=np.float32)  # [B, 2, NQ, O]
        interleaved = arr.transpose(0, 2, 1, 3).reshape(B, 2 * NQ, O)
        out[:, o0:o1, :] = interleaved[:, :ni, :]
    return out


def kernel(x, node_embeddings, laplacian_mx, cluster_weights_pool, bias_pool):
    in_maps = prepare_in_maps(
        x, node_embeddings, laplacian_mx, cluster_weights_pool, bias_pool
    )
    res = run_device(in_maps, trace=False)
    return assemble(res.results)
